# revision 22
# baseline (speedup 1.0000x reference)
"""Trainium2 Bass kernel for the BDH dense-transformer problem (v2.5).

Sharding: 8 cores = 4 heads x 2 block-groups: core c = (head c//2, group
c%2); group A owns canonical 128-row blocks {0,3,4,7}, B {1,2,5,6}. On-core
time order: own blocks at positions 0..3, peer at 4..7.

Numerics: fp16 everywhere except the score strips, which run fp8 DoubleRow
on rope outputs (error-tolerant because the E*E/O*O products correlate
positively). fp8 on the x/weights/ykv paths was measured at 3-6e-2 final
error (random-sign dots keep per-element quantization noise), so those
stay fp16.

Structure vs v1: y_sparse/xy/MLP run in two own-column halves, each with
its own 4-core AllReduce, so AR(half0) overlaps half1 compute and the
half0 LN epilogue overlaps AR(half1); x0 and x0^T arrive precomputed from
the host; ykv is fully normalized in one pass.
"""

import math

import numpy as np

P = 128
T = 1024
D = 192
NH = 4
N = 3072
NPAIR = 1536
NPC = 12          # 128-row chunks of the 1536 rope pairs
VOCAB = 256
EPS = 1e-5
N_LAYER = 4
NCORES = 8
HALF = 512
TBLK = T // P     # 8 canonical 128-row blocks
QUART = 256
WARM = 0          # zero-stationary PE warmer matmuls per lead-in chunk

_CACHE = {}

OWN_A = [0, 3, 4, 7]
OWN_B = [1, 2, 5, 6]


def _get_freqs(n, theta=2.0 ** 16):
    t = np.arange(n, dtype=np.float32)
    q = np.floor(t / 2.0) * 2.0
    return (1.0 / theta ** (q / n) / (2.0 * math.pi)).astype(np.float32)


def _ln_np(x):
    m = x.mean(-1, keepdims=True)
    v = x.var(-1, keepdims=True)
    return (x - m) / np.sqrt(v + EPS)


def build_program(repeat=1):
    key = ("nc", repeat)
    if key in _CACHE:
        return _CACHE[key]

    import concourse.mybir as mybir
    import concourse.tile as tile
    from concourse import bacc

    f16 = mybir.dt.float16
    f32 = mybir.dt.float32
    f8 = mybir.dt.float8e4
    AF = mybir.ActivationFunctionType
    OP = mybir.AluOpType
    DR = mybir.MatmulPerfMode.DoubleRow

    nc = bacc.Bacc("TRN2", target_bir_lowering=False, debug=False,
                   num_devices=NCORES)

    # ---- I/O ----
    x0_d = nc.dram_tensor("x0", [T, D], f16, kind="ExternalInput")
    x0t_d = nc.dram_tensor("x0t", [P, 2 * T], f16, kind="ExternalInput")
    wxe_d = nc.dram_tensor("wxe", [P, 2 * NPAIR], f16, kind="ExternalInput")
    wxo_d = nc.dram_tensor("wxo", [P, 2 * NPAIR], f16, kind="ExternalInput")
    wye_d = nc.dram_tensor("wye", [P, 2 * NPAIR], f16, kind="ExternalInput")
    wyo_d = nc.dram_tensor("wyo", [P, 2 * NPAIR], f16, kind="ExternalInput")
    ence_d = nc.dram_tensor("ence", [NPAIR, D], f16, kind="ExternalInput")
    enco_d = nc.dram_tensor("enco", [NPAIR, D], f16, kind="ExternalInput")
    cos_d = nc.dram_tensor("cosT", [NPAIR, T], f16, kind="ExternalInput")
    sin_d = nc.dram_tensor("sinT", [NPAIR, T], f16, kind="ExternalInput")
    m0_d = nc.dram_tensor("m0", [P, P], f16, kind="ExternalInput")
    gsel_d = nc.dram_tensor("gsel", [P, 2], f32, kind="ExternalInput")
    lmh2_d = nc.dram_tensor("lmh2", [P, 2 * VOCAB], f16, kind="ExternalInput")
    ident_d = nc.dram_tensor("ident", [P, P], f16, kind="ExternalInput")
    logits_d = nc.dram_tensor("logits", [HALF, VOCAB], f32, kind="ExternalOutput")

    GROUPS_AR = [[0, 2, 4, 6], [1, 3, 5, 7]]
    GROUPS_AG = [[0, 1], [2, 3], [4, 5], [6, 7]]

    with tile.TileContext(nc) as tc:
        with (
            tc.tile_pool(name="const", bufs=1) as cpool,
            tc.tile_pool(name="state", bufs=1) as spool,
            tc.tile_pool(name="work", bufs=2) as work,
            tc.tile_pool(name="stats", bufs=2) as stp,
            tc.tile_pool(name="psum", bufs=1, space="PSUM") as psp,
            tc.tile_pool(name="dram", bufs=1, space="DRAM") as dpool,
        ):
            # ---- persistent SBUF residents (k-tile packed: [:,0,:] = rows
            # 0:128 of the 192-row dim, [0:64,1,:] = rows 128:192) ----
            wxe = cpool.tile([P, 2, NPAIR], f16, tag="wxe")
            wxo = cpool.tile([P, 2, NPAIR], f16, tag="wxo")
            wye = cpool.tile([P, 2, NPAIR], f16, tag="wye")
            wyo = cpool.tile([P, 2, NPAIR], f16, tag="wyo")
            ence_t = cpool.tile([P, NPC, D], f16, tag="ence")
            enco_t = cpool.tile([P, NPC, D], f16, tag="enco")
            cos_t = cpool.tile([P, NPC, T], f16, tag="cos")
            sin_t = cpool.tile([P, NPC, T], f16, tag="sin")
            m0_t = cpool.tile([P, P], f16, tag="m0")
            gsel_t = cpool.tile([P, 2], f32, tag="gsel")
            eps_t = cpool.tile([P, 1], f32, tag="eps")
            ident_t = cpool.tile([P, P], f16, tag="ident")
            lmh2 = cpool.tile([P, 2, VOCAB], f16, tag="lmh2")

            qrE = spool.tile([P, NPC, T], f8, tag="qrE")
            qrO = spool.tile([P, NPC, T], f8, tag="qrO")
            Eown = spool.tile([P, NPC, HALF], f16, tag="Eown")
            Oown = spool.tile([P, NPC, HALF], f16, tag="Oown")
            x16 = spool.tile([P, TBLK, D], f16, tag="x16")
            xT = spool.tile([P, 2, T], f16, tag="xT")
            scT = spool.tile([P, TBLK, HALF], f16, tag="scT")
            ykvT = spool.tile([P, 2, HALF], f16, tag="ykvT")

            ar_in = [dpool.tile([2 * P, D], f16, name=f"ari{h}") for h in range(2)]
            ar_out = [dpool.tile([2 * P, D], f16, name=f"aro{h}") for h in range(2)]
            ag_in = dpool.tile([HALF, D], f16)
            ag_out = dpool.tile([T, D], f16)

            # ---- load constants; wave-0's needs first ----
            nc.sync.dma_start(ident_t[:, :], ident_d[:, :])
            nc.sync.dma_start(m0_t[:, :], m0_d[:, :])
            nc.sync.dma_start(gsel_t[:, :], gsel_d[:, :])
            for k in range(2):
                nc.sync.dma_start(xT[:, k, :], x0t_d[:, k * T:(k + 1) * T])
                nc.sync.dma_start(wxe[:, k, :], wxe_d[:, k * NPAIR:(k + 1) * NPAIR])
                nc.sync.dma_start(wxo[:, k, :], wxo_d[:, k * NPAIR:(k + 1) * NPAIR])
            for cb in range(TBLK):
                (nc.scalar if cb % 2 == 0 else nc.gpsimd).dma_start(
                    x16[:, cb, :], x0_d[cb * P:(cb + 1) * P, :])
            # warm up the collective channels EARLY (first collective pays
            # ~50us of setup; a pending trigger parks the gpsimd queue, so
            # only late-deadline DMAs may sit behind it there).
            warm_in = dpool.tile([2 * P, D], f16)
            warm_out = dpool.tile([2 * P, D], f16)
            wag_out = dpool.tile([HALF, D], f16)
            nc.scalar.dma_start(warm_in[0:P, 0:P], m0_t[:, :])
            nc.gpsimd.collective_compute(
                "AllReduce", OP.add, replica_groups=GROUPS_AR,
                ins=[warm_in.opt()], outs=[warm_out.opt()])
            nc.gpsimd.collective_compute(
                "AllGather", OP.bypass, replica_groups=GROUPS_AG,
                ins=[warm_out.opt()], outs=[wag_out.opt()])
            # encoder tiles are only needed ~95us in; they ride the parked
            # gpsimd queue. Everything else streams on the sync queue in
            # deadline order (cos/sin own half, peer half, y-weights, lm).
            for pc in range(NPC):
                nc.gpsimd.dma_start(ence_t[:, pc, :], ence_d[pc * P:(pc + 1) * P, :])
                nc.gpsimd.dma_start(enco_t[:, pc, :], enco_d[pc * P:(pc + 1) * P, :])
            for pc in range(NPC):
                nc.sync.dma_start(cos_t[:, pc, 0:HALF], cos_d[pc * P:(pc + 1) * P, 0:HALF])
                nc.sync.dma_start(sin_t[:, pc, 0:HALF], sin_d[pc * P:(pc + 1) * P, 0:HALF])
            for pc in range(NPC):
                nc.sync.dma_start(cos_t[:, pc, HALF:T], cos_d[pc * P:(pc + 1) * P, HALF:T])
                nc.sync.dma_start(sin_t[:, pc, HALF:T], sin_d[pc * P:(pc + 1) * P, HALF:T])
            for k in range(2):
                nc.sync.dma_start(wye[:, k, :], wye_d[:, k * NPAIR:(k + 1) * NPAIR])
                nc.sync.dma_start(wyo[:, k, :], wyo_d[:, k * NPAIR:(k + 1) * NPAIR])
            for k in range(2):
                nc.sync.dma_start(lmh2[:, k, :], lmh2_d[:, k * VOCAB:(k + 1) * VOCAB])

            nc.vector.memset(eps_t[:, :], EPS)
            nc.vector.memset(ykvT[64:P, 1, :], 0)
            zer8 = cpool.tile([P, 2, P], f8, tag="zer8")
            nc.vector.memset(zer8[:, :, :], 0)

            # zero regions of masked score strips (stay zero forever)
            for s in (1, 2, 3, 5, 6, 7):
                nc.vector.memset(scT[:, s, 0:(s % 4) * P], 0)

            def mm_pair(out_ap, w, pcs, mov, msl, start=True, stop=True):
                """fp16 contract-192 pair: k-tile 0 (128 rows) + k-tile 1
                (64 rows) of stationary w[:, k, pcs] x moving mov[:, k, msl]."""
                nc.tensor.matmul(out_ap, w[:, 0, pcs], mov[:, 0, msl],
                                 start=start, stop=False)
                nc.tensor.matmul(out_ap, w[0:64, 1, pcs], mov[0:64, 1, msl],
                                 start=False, stop=stop)

            def pe_transpose(src_ap_full, col, tagp):
                """[128, 192] fp16 tile -> xT k-tiles at column col."""
                tp0 = psp.tile([P, P], f16, tag="xspO", bufs=2, name=f"tp0_{tagp}")
                nc.tensor.transpose(tp0[:, :], src_ap_full[:, 0:P], ident_t[:, :])
                nc.scalar.copy(xT[:, 0, col:col + P], tp0[:, :])
                tp1 = psp.tile([P, P], f16, tag="xspO", bufs=2, name=f"tp1_{tagp}")
                nc.tensor.transpose(tp1[0:64, :], src_ap_full[:, P:D], ident_t[:, :])
                nc.scalar.copy(xT[0:64, 1, col:col + P], tp1[0:64, :])

            def bn_stats2(src3d, tagx):
                """per-slot LN stats over 2 slots -> rv, nmr [P, 2] f32."""
                mv = stp.tile([P, 2, 2], f32, tag="mvB", name=f"mv{tagx}")
                for j in range(2):
                    st = stp.tile([P, 6], f32, tag="bnstB", bufs=2, name=f"bst{tagx}")
                    nc.vector.bn_stats(st[:, :], src3d[:, j, :])
                    nc.vector.bn_aggr(mv[:, j, :], st[:, :])
                sd = stp.tile([P, 2], f32, tag="sdBN", name=f"sdB{tagx}")
                nc.scalar.activation(sd[:, :], mv[:, :, 1], AF.Sqrt, bias=eps_t[:, :])
                rv = stp.tile([P, 2], f32, tag="rvBN", name=f"rvB{tagx}")
                nc.vector.reciprocal(rv[:, :], sd[:, :])
                nmr = stp.tile([P, 2], f32, tag="nmrBN", name=f"nmrB{tagx}")
                nc.vector.scalar_tensor_tensor(nmr[:, :], mv[:, :, 0], -1.0, rv[:, :],
                                               OP.mult, OP.mult)
                return rv, nmr

            def lm_head_tb(tb):
                """own block tb x full vocab; runs inside the last epilogue."""
                tbs = slice(tb * P, (tb + 1) * P)
                psL = psp.tile([P, HALF], f32, tag="xspO", bufs=2, name=f"psL{tb}")
                nc.tensor.matmul(psL[:, 0:VOCAB], xT[:, 0, tbs], lmh2[:, 0, :],
                                 start=True, stop=False)
                nc.tensor.matmul(psL[:, 0:VOCAB], xT[0:64, 1, tbs], lmh2[0:64, 1, :],
                                 start=False, stop=True)
                outL = work.tile([P, VOCAB], f32, tag="outL", bufs=2)
                nc.scalar.copy(outL[:, :], psL[:, 0:VOCAB])
                (nc.sync if tb % 2 == 0 else nc.gpsimd).dma_start(
                    logits_d[tbs, :], outL[:, :])

            ho_tiles = {}

            def fetch_other(li):
                Ho0 = work.tile([P, 4, D], f16, tag="Ho0", bufs=1, name=f"Ho0_{li}")
                Ho1 = work.tile([P, 4, D], f16, tag="Ho1", bufs=1, name=f"Ho1_{li}")
                for j in range(4):
                    nc.sync.dma_start(Ho0[:, j, :], ag_out[j * P:(j + 1) * P, :])
                    nc.sync.dma_start(Ho1[:, j, :], ag_out[HALF + j * P:HALF + (j + 1) * P, :])
                ho_tiles[li] = (Ho0, Ho1)

            def combine_other(li):
                Ho0, Ho1 = ho_tiles.pop(li - 1)
                tmp = work.tile([P, 4, D], f16, tag="HoT", bufs=1, name=f"HoT_{li}")
                nc.scalar.mul(tmp[:, :, :], Ho0[:, :, :], gsel_t[:, 1:2])
                nc.vector.scalar_tensor_tensor(
                    x16[:, 4:8, :], Ho1[:, :, :], gsel_t[:, 0:1], tmp[:, :, :],
                    OP.mult, OP.add)
                for j in range(4):
                    pe_transpose(x16[:, 4 + j, :], (4 + j) * P, f"o{li}{j}")

            def wave(wv, li):
                """x_sparse (fp16) + rope (fp16 DVE, fp8 out) + fp8 DR score
                strips for one column half."""
                LAGP = 4
                s_lo = wv * 4
                tsl = slice(wv * HALF, (wv + 1) * HALF)

                def sc_mms(ps_list, pcp):
                    psl = slice(2 * pcp, 2 * pcp + 2)
                    for si, psS in enumerate(ps_list):
                        s = s_lo + si
                        ssl = slice(s * P, (s + 1) * P)
                        osl = slice(si * P, HALF)  # causal trim (both waves)
                        nc.tensor.matmul(psS[:, osl], qrE[:, psl, ssl], qrE[:, psl, osl],
                                         start=(pcp == 0), stop=False,
                                         perf_mode=DR)
                        nc.tensor.matmul(psS[:, osl], qrO[:, psl, ssl], qrO[:, psl, osl],
                                         start=False, stop=(pcp == NPC // 2 - 1),
                                         perf_mode=DR)

                ps_list = []
                for si in range(4):
                    psS = psp.tile([P, HALF], f32, tag=f"sc{si}",
                                   name=f"psS{li}_{s_lo + si}")
                    ps_list.append(psS)
                for kk in range(NPC // 2):
                    if wv == 0:
                        Et2 = Eown[:, 2 * kk:2 * kk + 2, :]
                        Ot2 = Oown[:, 2 * kk:2 * kk + 2, :]
                    else:
                        Ew = work.tile([P, 2, HALF], f16, tag="E1")
                        Ow = work.tile([P, 2, HALF], f16, tag="O1")
                        Et2 = Ew[:, :, :]
                        Ot2 = Ow[:, :, :]
                    for k in range(2):
                        pc = 2 * kk + k
                        pcs = slice(pc * P, (pc + 1) * P)
                        psE = psp.tile([P, HALF], f32, tag="xspE", bufs=2)
                        psO = psp.tile([P, HALF], f32, tag="xspO", bufs=2)
                        if wv == 0:
                            # split the first chunks by column half so PE can
                            # start as soon as the h0 epilogue lands, instead
                            # of gating on the h1 AllReduce readback
                            for ch in range(2):
                                csl2 = slice(ch * QUART, (ch + 1) * QUART)
                                nc.tensor.matmul(psE[:, csl2], wxe[:, 0, pcs],
                                                 xT[:, 0, csl2], start=True, stop=False)
                                nc.tensor.matmul(psE[:, csl2], wxe[0:64, 1, pcs],
                                                 xT[0:64, 1, csl2], start=False, stop=True)
                                nc.tensor.matmul(psO[:, csl2], wxo[:, 0, pcs],
                                                 xT[:, 0, csl2], start=True, stop=False)
                                nc.tensor.matmul(psO[:, csl2], wxo[0:64, 1, pcs],
                                                 xT[0:64, 1, csl2], start=False, stop=True)
                        else:
                            mm_pair(psE[:, :], wxe, pcs, xT, tsl)
                            mm_pair(psO[:, :], wxo, pcs, xT, tsl)
                        nc.scalar.activation(Et2[:, k, :], psE[:, :], AF.Relu)
                        nc.scalar.activation(Ot2[:, k, :], psO[:, :], AF.Relu)
                    psl2 = slice(2 * kk, 2 * kk + 2)
                    cs = cos_t[:, psl2, tsl]
                    sn = sin_t[:, psl2, tsl]
                    t1 = work.tile([P, 2, HALF], f16, tag="rt1")
                    t2 = work.tile([P, 2, HALF], f16, tag="rt2")
                    nc.vector.tensor_mul(t1[:, :, :], Et2, cs)
                    nc.vector.tensor_mul(t2[:, :, :], Ot2, sn)
                    nc.vector.tensor_sub(qrE[:, psl2, tsl], t1[:, :, :], t2[:, :, :])
                    t3 = work.tile([P, 2, HALF], f16, tag="rt3")
                    t4 = work.tile([P, 2, HALF], f16, tag="rt4")
                    nc.vector.tensor_mul(t3[:, :, :], Ot2, cs)
                    nc.vector.tensor_mul(t4[:, :, :], Et2, sn)
                    nc.vector.tensor_add(qrO[:, psl2, tsl], t3[:, :, :], t4[:, :, :])
                    if kk >= LAGP:
                        sc_mms(ps_list, kk - LAGP)
                    elif kk > 0:
                        # zero-stationary warmers: keep the PE clock ramped
                        # through the DVE-paced lead-in (adds 0 to a live
                        # strip accumulation, so results are unchanged)
                        for _w in range(WARM):
                            nc.tensor.matmul(ps_list[0][:, 0:HALF],
                                             zer8[:, :, 0:P],
                                             qrE[:, 0:2, 0:HALF],
                                             start=False, stop=False,
                                             perf_mode=DR)
                for pcp in range(NPC // 2 - LAGP, NPC // 2):
                    sc_mms(ps_list, pcp)
                for si in range(4):
                    s = s_lo + si
                    psS = ps_list[si]
                    dsl = slice(si * P, (si + 1) * P)
                    if wv == 0:
                        nc.vector.tensor_tensor(scT[:, s, dsl], psS[:, dsl],
                                                m0_t[:, :], OP.mult)
                    else:
                        sel = gsel_t[:, 0:1] if si % 2 == 1 else gsel_t[:, 1:2]
                        nc.scalar.mul(scT[:, s, dsl], psS[:, dsl], sel)
                    if si < 3:
                        csl = slice((si + 1) * P, HALF)
                        nc.scalar.copy(scT[:, s, csl], psS[:, csl])

            def ykv_block(tb, li):
                """scores @ x for own block tb, full LN, transpose to ykvT."""
                tbs = slice(tb * P, (tb + 1) * P)
                psY = psp.tile([P, D], f32, tag="xspE", bufs=2)
                s_list = [s for s in range(TBLK) if s % 4 <= tb]
                for s in s_list:
                    nc.tensor.matmul(psY[:, :], scT[:, s, tbs], x16[:, s, :],
                                     start=(s == s_list[0]), stop=(s == s_list[-1]))
                st = stp.tile([P, 6], f32, tag="bnst")
                nc.vector.bn_stats(st[:, :], psY[:, :])
                mv = stp.tile([P, 2], f32, tag="bnmv")
                nc.vector.bn_aggr(mv[:, :], st[:, :])
                sd = stp.tile([P, 1], f32, tag=f"sdk{tb}")
                nc.scalar.activation(sd[:, :], mv[:, 1:2], AF.Sqrt, bias=eps_t[:, :])
                rk = stp.tile([P, 1], f32, tag=f"rk{tb}")
                nc.vector.reciprocal(rk[:, :], sd[:, :])
                nmr = stp.tile([P, 1], f32, tag=f"nmk{tb}")
                nc.vector.scalar_tensor_tensor(nmr[:, :], mv[:, 0:1], -1.0, rk[:, :],
                                               OP.mult, OP.mult)
                ykvn = work.tile([P, D], f16, tag="ykvn", bufs=2)
                nc.scalar.activation(ykvn[:, :], psY[:, :], AF.Identity,
                                     bias=nmr[:, :], scale=rk[:, :])
                tp0 = psp.tile([P, P], f16, tag="xspO", bufs=2, name=f"ytp0_{li}{tb}")
                nc.tensor.transpose(tp0[:, :], ykvn[:, 0:P], ident_t[:, :])
                nc.scalar.copy(ykvT[:, 0, tbs], tp0[:, :])
                tp1 = psp.tile([P, P], f16, tag="xspO", bufs=2, name=f"ytp1_{li}{tb}")
                nc.tensor.transpose(tp1[0:64, :], ykvn[:, P:D], ident_t[:, :])
                nc.scalar.copy(ykvT[0:64, 1, tbs], tp1[0:64, :])

            def layer(li):
                wave(0, li)
                if li > 0:
                    combine_other(li)
                wave(1, li)

                for tb in range(4):
                    ykv_block(tb, li)

                # ---- y_sparse / xy / mlp in two own-column halves ----
                psM = []
                for _mi in range(4):
                    psM_t = psp.tile([P, D], f32, tag=f"sc{_mi}", name=f"psM{li}_{_mi}")
                    psM.append(psM_t)
                for h in range(2):
                    osl = slice(h * QUART, (h + 1) * QUART)
                    for side in range(2):
                        wa = wye if side == 0 else wyo
                        own = Eown if side == 0 else Oown
                        enc_t = ence_t if side == 0 else enco_t
                        for kk in range(NPC // 2):
                            ys2 = work.tile([P, 2, QUART], f16, tag="ys")
                            for k in range(2):
                                pc = 2 * kk + k
                                pcs = slice(pc * P, (pc + 1) * P)
                                psYS = psp.tile([P, QUART], f32, tag="xspE", bufs=2)
                                mm_pair(psYS[:, :], wa, pcs, ykvT, osl)
                                # split relus ACT/DVE: shortens the h phase,
                                # which gates this half's AllReduce trigger
                                if pc % 3 == 2:
                                    nc.vector.tensor_scalar_max(ys2[:, k, :],
                                                                psYS[:, :], 0.0)
                                else:
                                    nc.scalar.activation(ys2[:, k, :], psYS[:, :],
                                                         AF.Relu)
                            xy2 = work.tile([P, 2, QUART], f16, tag="xy", bufs=3)
                            nc.vector.tensor_mul(xy2[:, :, :],
                                                 own[:, 2 * kk:2 * kk + 2, osl],
                                                 ys2[:, :, :])
                            for k in range(2):
                                pc = 2 * kk + k
                                last = (side == 1 and pc == NPC - 1)
                                for j in range(2):
                                    tb = 2 * h + j
                                    nc.tensor.matmul(psM[tb][:, :],
                                                     xy2[:, k, j * P:(j + 1) * P],
                                                     enc_t[:, pc, :],
                                                     start=(side == 0 and pc == 0),
                                                     stop=last)
                    # AllReduce of this half's MLP partial over the 4 heads
                    for j in range(2):
                        tb = 2 * h + j
                        bA = work.tile([P, D], f16, tag="bA", bufs=2, name=f"bA{j}")
                        if j == 0:
                            nc.scalar.copy(bA[:, :], psM[tb][:, :])
                        else:
                            nc.vector.tensor_copy(bA[:, :], psM[tb][:, :])
                        (nc.sync if j == 0 else nc.scalar).dma_start(
                            ar_in[h][j * P:(j + 1) * P, :], bA[:, :])
                    nc.gpsimd.collective_compute(
                        "AllReduce", OP.add,
                        replica_groups=GROUPS_AR if (li + h) % 2 == 0 else GROUPS_AR[::-1],
                        ins=[ar_in[h].opt()],
                        outs=[ar_out[h].opt()],
                    )

                # ---- per-half readback, ln(ymlp), residual, ln, new x ----
                for h in range(2):
                    Hall = work.tile([P, 2, D], f16, tag=f"Hall{h}", bufs=1,
                                     name=f"Hall{li}_{h}")
                    rb_q = [nc.sync, nc.scalar]
                    for j in range(2):
                        rb_q[j].dma_start(Hall[:, j, :], ar_out[h][j * P:(j + 1) * P, :])
                    rv, nmr = bn_stats2(Hall[:, :, :], f"H{li}{h}")
                    t1a = work.tile([P, 2, D], f32, tag="t1a", bufs=2)
                    for j in range(2):
                        nc.scalar.activation(t1a[:, j, :], Hall[:, j, :], AF.Identity,
                                             bias=nmr[:, j:j + 1], scale=rv[:, j:j + 1])
                    XM = work.tile([P, 2, D], f16, tag=f"XM{h}", bufs=1,
                                   name=f"XM{li}_{h}")
                    nc.vector.tensor_add(XM[:, :, :], t1a[:, :, :],
                                         x16[:, 2 * h:2 * h + 2, :])
                    rv2, nm2 = bn_stats2(XM[:, :, :], f"X{li}{h}")
                    for j in range(2):
                        slot = 2 * h + j
                        nc.scalar.activation(x16[:, slot, :], XM[:, j, :], AF.Identity,
                                             bias=nm2[:, j:j + 1], scale=rv2[:, j:j + 1])
                        if li < N_LAYER - 1:
                            (nc.gpsimd if slot % 2 == 0 else nc.sync).dma_start(
                                ag_in[slot * P:(slot + 1) * P, :], x16[:, slot, :])
                        pe_transpose(x16[:, slot, :], slot * P, f"n{li}{slot}")
                        if li == N_LAYER - 1:
                            lm_head_tb(slot)

                # ---- pair-exchange of the finished own half ----
                if li < N_LAYER - 1:
                    nc.gpsimd.collective_compute(
                        "AllGather", OP.bypass,
                        replica_groups=GROUPS_AG[li % 4:] + GROUPS_AG[:li % 4],
                        ins=[ag_in.opt()],
                        outs=[ag_out.opt()],
                    )
                    fetch_other(li)

            for rep in range(repeat):
                for li in range(N_LAYER):
                    layer(li)

    nc.compile()
    _CACHE[key] = nc
    return nc


def _pack_ktiles16(w):
    """[D, C] -> [128, 2, C] fp16 k-tile layout (rows 0:128 | 128:192+pad)."""
    c = w.shape[1]
    out = np.zeros((P, 2, c), dtype=np.float16)
    out[:, 0, :] = w[0:P].astype(np.float16)
    out[0:64, 1, :] = w[P:D].astype(np.float16)
    return out.reshape(P, 2 * c)


def make_inputs(idx, decoder_x, decoder_y, encoder, embed, pos_emb, lm_head):
    """Host-side prep: per-core input dicts (core c = head c//2, group c%2)."""
    idx = np.asarray(idx)
    decoder_x = np.asarray(decoder_x, dtype=np.float32)
    decoder_y = np.asarray(decoder_y, dtype=np.float32)
    encoder = np.asarray(encoder, dtype=np.float32).reshape(NH, N, D)
    embed = np.asarray(embed, dtype=np.float32)
    pos_emb = np.asarray(pos_emb, dtype=np.float32)
    lm_head = np.asarray(lm_head, dtype=np.float32)

    x0f = _ln_np(embed[idx[0]] + pos_emb[:T]).astype(np.float32)

    freqs = _get_freqs(N)
    fpair = freqs[0::2]
    tt = np.arange(T, dtype=np.float32)
    m0 = np.triu(np.ones((P, P), np.float32), k=1).astype(np.float16)

    lmh2 = np.zeros((P, 2, VOCAB), np.float16)
    lmh2[:, 0, :] = lm_head[0:P].astype(np.float16)
    lmh2[0:64, 1, :] = lm_head[P:D].astype(np.float16)
    lmh2 = lmh2.reshape(P, 2 * VOCAB)

    in_maps = []
    for c in range(NCORES):
        h, g = c // 2, c % 2
        own = OWN_A if g == 0 else OWN_B
        peer = OWN_B if g == 0 else OWN_A
        tsel = np.concatenate([np.arange(b * P, (b + 1) * P) for b in own + peer])
        tperm = tt[tsel]
        ph = ((fpair[:, None] * tperm[None, :]).astype(np.float32) % 1.0) \
            * np.float32(2.0 * math.pi)
        gsel = np.zeros((P, 2), np.float32)
        gsel[:, 0] = 1.0 if g == 0 else 0.0
        gsel[:, 1] = 1.0 - gsel[:, 0]
        x0c = x0f[tsel]
        in_maps.append({
            "x0": x0c.astype(np.float16),
            "x0t": _pack_ktiles16(x0c.T),
            "wxe": _pack_ktiles16(np.ascontiguousarray(decoder_x[h][:, 0::2])),
            "wxo": _pack_ktiles16(np.ascontiguousarray(decoder_x[h][:, 1::2])),
            "wye": _pack_ktiles16(np.ascontiguousarray(decoder_y[h][:, 0::2])),
            "wyo": _pack_ktiles16(np.ascontiguousarray(decoder_y[h][:, 1::2])),
            "ence": np.ascontiguousarray(encoder[h][0::2]).astype(np.float16),
            "enco": np.ascontiguousarray(encoder[h][1::2]).astype(np.float16),
            "cosT": np.cos(ph.astype(np.float64)).astype(np.float16),
            "sinT": np.sin(ph.astype(np.float64)).astype(np.float16),
            "m0": m0,
            "gsel": gsel,
            "lmh2": lmh2,
            "ident": np.eye(P, dtype=np.float16),
        })
    return in_maps


def kernel(idx, decoder_x, decoder_y, encoder, embed, pos_emb, lm_head):
    from concourse.bass_utils import run_bass_kernel_spmd

    nc = build_program()
    in_maps = make_inputs(idx, decoder_x, decoder_y, encoder, embed, pos_emb,
                          lm_head)
    res = run_bass_kernel_spmd(nc, in_maps, list(range(NCORES)))
    return assemble_logits(res.results)


def assemble_logits(results):
    logits = np.empty((T, VOCAB), np.float32)
    for c in (0, 1):
        own = OWN_A if c % 2 == 0 else OWN_B
        sl = results[c]["logits"]
        for pos, b in enumerate(own):
            logits[b * P:(b + 1) * P] = sl[pos * P:(pos + 1) * P]
    return logits.reshape(1, T, VOCAB).astype(np.float32)


# revision 23
# speedup vs baseline: 1.1909x; 1.1909x over previous
"""Trainium2 Bass kernel for the BDH dense-transformer problem (v2.5).

Sharding: 8 cores = 4 heads x 2 block-groups: core c = (head c//2, group
c%2); group A owns canonical 128-row blocks {0,3,4,7}, B {1,2,5,6}. On-core
time order: own blocks at positions 0..3, peer at 4..7.

Numerics: fp16 everywhere except the score strips, which run fp8 DoubleRow
on rope outputs (error-tolerant because the E*E/O*O products correlate
positively). fp8 on the x/weights/ykv paths was measured at 3-6e-2 final
error (random-sign dots keep per-element quantization noise), so those
stay fp16.

Structure vs v1: y_sparse/xy/MLP run in two own-column halves, each with
its own 4-core AllReduce, so AR(half0) overlaps half1 compute and the
half0 LN epilogue overlaps AR(half1); x0 and x0^T arrive precomputed from
the host; ykv is fully normalized in one pass.
"""

import math

import numpy as np

P = 128
T = 1024
D = 192
NH = 4
N = 3072
NPAIR = 1536
NPC = 12          # 128-row chunks of the 1536 rope pairs
VOCAB = 256
EPS = 1e-5
N_LAYER = 4
NCORES = 8
HALF = 512
TBLK = T // P     # 8 canonical 128-row blocks
QUART = 256
WARM = 2          # zero-stationary PE warmer matmuls per lead-in chunk

_CACHE = {}

OWN_A = [0, 3, 4, 7]
OWN_B = [1, 2, 5, 6]


def _get_freqs(n, theta=2.0 ** 16):
    t = np.arange(n, dtype=np.float32)
    q = np.floor(t / 2.0) * 2.0
    return (1.0 / theta ** (q / n) / (2.0 * math.pi)).astype(np.float32)


def _ln_np(x):
    m = x.mean(-1, keepdims=True)
    v = x.var(-1, keepdims=True)
    return (x - m) / np.sqrt(v + EPS)


def build_program(repeat=1):
    key = ("nc", repeat)
    if key in _CACHE:
        return _CACHE[key]

    import concourse.mybir as mybir
    import concourse.tile as tile
    from concourse import bacc

    f16 = mybir.dt.float16
    f32 = mybir.dt.float32
    f8 = mybir.dt.float8e4
    AF = mybir.ActivationFunctionType
    OP = mybir.AluOpType
    DR = mybir.MatmulPerfMode.DoubleRow

    nc = bacc.Bacc("TRN2", target_bir_lowering=False, debug=False,
                   num_devices=NCORES)

    # ---- I/O ----
    x0_d = nc.dram_tensor("x0", [T, D], f16, kind="ExternalInput")
    x0t_d = nc.dram_tensor("x0t", [P, 2 * T], f16, kind="ExternalInput")
    wxe_d = nc.dram_tensor("wxe", [P, 2 * NPAIR], f16, kind="ExternalInput")
    wxo_d = nc.dram_tensor("wxo", [P, 2 * NPAIR], f16, kind="ExternalInput")
    wye_d = nc.dram_tensor("wye", [P, 2 * NPAIR], f16, kind="ExternalInput")
    wyo_d = nc.dram_tensor("wyo", [P, 2 * NPAIR], f16, kind="ExternalInput")
    ence_d = nc.dram_tensor("ence", [NPAIR, D], f16, kind="ExternalInput")
    enco_d = nc.dram_tensor("enco", [NPAIR, D], f16, kind="ExternalInput")
    cos_d = nc.dram_tensor("cosT", [NPAIR, T], f16, kind="ExternalInput")
    sin_d = nc.dram_tensor("sinT", [NPAIR, T], f16, kind="ExternalInput")
    m0_d = nc.dram_tensor("m0", [P, P], f16, kind="ExternalInput")
    gsel_d = nc.dram_tensor("gsel", [P, 2], f32, kind="ExternalInput")
    lmh2_d = nc.dram_tensor("lmh2", [P, 2 * VOCAB], f16, kind="ExternalInput")
    ident_d = nc.dram_tensor("ident", [P, P], f16, kind="ExternalInput")
    logits_d = nc.dram_tensor("logits", [HALF, VOCAB], f32, kind="ExternalOutput")

    GROUPS_AR = [[0, 2, 4, 6], [1, 3, 5, 7]]
    GROUPS_AG = [[0, 1], [2, 3], [4, 5], [6, 7]]

    with tile.TileContext(nc) as tc:
        with (
            tc.tile_pool(name="const", bufs=1) as cpool,
            tc.tile_pool(name="state", bufs=1) as spool,
            tc.tile_pool(name="work", bufs=2) as work,
            tc.tile_pool(name="stats", bufs=2) as stp,
            tc.tile_pool(name="psum", bufs=1, space="PSUM") as psp,
            tc.tile_pool(name="dram", bufs=1, space="DRAM") as dpool,
        ):
            # ---- persistent SBUF residents (k-tile packed: [:,0,:] = rows
            # 0:128 of the 192-row dim, [0:64,1,:] = rows 128:192) ----
            wxe = cpool.tile([P, 2, NPAIR], f16, tag="wxe")
            wxo = cpool.tile([P, 2, NPAIR], f16, tag="wxo")
            wye = cpool.tile([P, 2, NPAIR], f16, tag="wye")
            wyo = cpool.tile([P, 2, NPAIR], f16, tag="wyo")
            ence_t = cpool.tile([P, NPC, D], f16, tag="ence")
            enco_t = cpool.tile([P, NPC, D], f16, tag="enco")
            cos_t = cpool.tile([P, NPC, T], f16, tag="cos")
            sin_t = cpool.tile([P, NPC, T], f16, tag="sin")
            m0_t = cpool.tile([P, P], f16, tag="m0")
            gsel_t = cpool.tile([P, 2], f32, tag="gsel")
            eps_t = cpool.tile([P, 1], f32, tag="eps")
            ident_t = cpool.tile([P, P], f16, tag="ident")
            lmh2 = cpool.tile([P, 2, VOCAB], f16, tag="lmh2")

            qrE = spool.tile([P, NPC, T], f8, tag="qrE")
            qrO = spool.tile([P, NPC, T], f8, tag="qrO")
            Eown = spool.tile([P, NPC, HALF], f16, tag="Eown")
            Oown = spool.tile([P, NPC, HALF], f16, tag="Oown")
            x16 = spool.tile([P, TBLK, D], f16, tag="x16")
            xT = spool.tile([P, 2, T], f16, tag="xT")
            scT = spool.tile([P, TBLK, HALF], f16, tag="scT")
            ykvT = spool.tile([P, 2, HALF], f16, tag="ykvT")

            ar_in = [dpool.tile([2 * P, D], f16, name=f"ari{h}") for h in range(2)]
            ar_out = [dpool.tile([2 * P, D], f16, name=f"aro{h}") for h in range(2)]
            ag_in = dpool.tile([HALF, D], f16)
            ag_out = dpool.tile([T, D], f16)

            # ---- load constants; wave-0's needs first ----
            nc.sync.dma_start(ident_t[:, :], ident_d[:, :])
            nc.sync.dma_start(m0_t[:, :], m0_d[:, :])
            nc.sync.dma_start(gsel_t[:, :], gsel_d[:, :])
            for k in range(2):
                nc.sync.dma_start(xT[:, k, :], x0t_d[:, k * T:(k + 1) * T])
                nc.sync.dma_start(wxe[:, k, :], wxe_d[:, k * NPAIR:(k + 1) * NPAIR])
                nc.sync.dma_start(wxo[:, k, :], wxo_d[:, k * NPAIR:(k + 1) * NPAIR])
            for cb in range(TBLK):
                (nc.scalar if cb % 2 == 0 else nc.gpsimd).dma_start(
                    x16[:, cb, :], x0_d[cb * P:(cb + 1) * P, :])
            # warm up the collective channels EARLY (first collective pays
            # ~50us of setup; a pending trigger parks the gpsimd queue, so
            # only late-deadline DMAs may sit behind it there).
            warm_in = dpool.tile([2 * P, D], f16)
            warm_out = dpool.tile([2 * P, D], f16)
            wag_out = dpool.tile([HALF, D], f16)
            nc.scalar.dma_start(warm_in[0:P, 0:P], m0_t[:, :])
            nc.gpsimd.collective_compute(
                "AllReduce", OP.add, replica_groups=GROUPS_AR,
                ins=[warm_in.opt()], outs=[warm_out.opt()])
            nc.gpsimd.collective_compute(
                "AllGather", OP.bypass, replica_groups=GROUPS_AG,
                ins=[warm_out.opt()], outs=[wag_out.opt()])
            # encoder tiles are only needed ~95us in; they ride the parked
            # gpsimd queue. Everything else streams on the sync queue in
            # deadline order (cos/sin own half, peer half, y-weights, lm).
            for pc in range(NPC):
                nc.gpsimd.dma_start(ence_t[:, pc, :], ence_d[pc * P:(pc + 1) * P, :])
                nc.gpsimd.dma_start(enco_t[:, pc, :], enco_d[pc * P:(pc + 1) * P, :])
            for pc in range(NPC):
                nc.sync.dma_start(cos_t[:, pc, 0:HALF], cos_d[pc * P:(pc + 1) * P, 0:HALF])
                nc.sync.dma_start(sin_t[:, pc, 0:HALF], sin_d[pc * P:(pc + 1) * P, 0:HALF])
            for pc in range(NPC):
                nc.sync.dma_start(cos_t[:, pc, HALF:T], cos_d[pc * P:(pc + 1) * P, HALF:T])
                nc.sync.dma_start(sin_t[:, pc, HALF:T], sin_d[pc * P:(pc + 1) * P, HALF:T])
            for k in range(2):
                nc.sync.dma_start(wye[:, k, :], wye_d[:, k * NPAIR:(k + 1) * NPAIR])
                nc.sync.dma_start(wyo[:, k, :], wyo_d[:, k * NPAIR:(k + 1) * NPAIR])
            for k in range(2):
                nc.sync.dma_start(lmh2[:, k, :], lmh2_d[:, k * VOCAB:(k + 1) * VOCAB])

            nc.vector.memset(eps_t[:, :], EPS)
            nc.vector.memset(ykvT[64:P, 1, :], 0)
            zer8 = cpool.tile([P, 2, P], f8, tag="zer8")
            nc.vector.memset(zer8[:, :, :], 0)

            # zero regions of masked score strips (stay zero forever)
            for s in (1, 2, 3, 5, 6, 7):
                nc.vector.memset(scT[:, s, 0:(s % 4) * P], 0)

            def mm_pair(out_ap, w, pcs, mov, msl, start=True, stop=True):
                """fp16 contract-192 pair: k-tile 0 (128 rows) + k-tile 1
                (64 rows) of stationary w[:, k, pcs] x moving mov[:, k, msl]."""
                nc.tensor.matmul(out_ap, w[:, 0, pcs], mov[:, 0, msl],
                                 start=start, stop=False)
                nc.tensor.matmul(out_ap, w[0:64, 1, pcs], mov[0:64, 1, msl],
                                 start=False, stop=stop)

            def pe_transpose(src_ap_full, col, tagp):
                """[128, 192] fp16 tile -> xT k-tiles at column col."""
                tp0 = psp.tile([P, P], f16, tag="xspO", bufs=2, name=f"tp0_{tagp}")
                nc.tensor.transpose(tp0[:, :], src_ap_full[:, 0:P], ident_t[:, :])
                nc.scalar.copy(xT[:, 0, col:col + P], tp0[:, :])
                tp1 = psp.tile([P, P], f16, tag="xspO", bufs=2, name=f"tp1_{tagp}")
                nc.tensor.transpose(tp1[0:64, :], src_ap_full[:, P:D], ident_t[:, :])
                nc.scalar.copy(xT[0:64, 1, col:col + P], tp1[0:64, :])

            def bn_stats2(src3d, tagx):
                """per-slot LN stats over 2 slots -> rv, nmr [P, 2] f32."""
                mv = stp.tile([P, 2, 2], f32, tag="mvB", name=f"mv{tagx}")
                for j in range(2):
                    st = stp.tile([P, 6], f32, tag="bnstB", bufs=2, name=f"bst{tagx}")
                    nc.vector.bn_stats(st[:, :], src3d[:, j, :])
                    nc.vector.bn_aggr(mv[:, j, :], st[:, :])
                sd = stp.tile([P, 2], f32, tag="sdBN", name=f"sdB{tagx}")
                nc.scalar.activation(sd[:, :], mv[:, :, 1], AF.Sqrt, bias=eps_t[:, :])
                rv = stp.tile([P, 2], f32, tag="rvBN", name=f"rvB{tagx}")
                nc.vector.reciprocal(rv[:, :], sd[:, :])
                nmr = stp.tile([P, 2], f32, tag="nmrBN", name=f"nmrB{tagx}")
                nc.vector.scalar_tensor_tensor(nmr[:, :], mv[:, :, 0], -1.0, rv[:, :],
                                               OP.mult, OP.mult)
                return rv, nmr

            def lm_head_tb(tb):
                """own block tb x full vocab; runs inside the last epilogue."""
                tbs = slice(tb * P, (tb + 1) * P)
                psL = psp.tile([P, HALF], f32, tag="xspO", bufs=2, name=f"psL{tb}")
                nc.tensor.matmul(psL[:, 0:VOCAB], xT[:, 0, tbs], lmh2[:, 0, :],
                                 start=True, stop=False)
                nc.tensor.matmul(psL[:, 0:VOCAB], xT[0:64, 1, tbs], lmh2[0:64, 1, :],
                                 start=False, stop=True)
                outL = work.tile([P, VOCAB], f32, tag="outL", bufs=2)
                nc.scalar.copy(outL[:, :], psL[:, 0:VOCAB])
                (nc.sync if tb % 2 == 0 else nc.gpsimd).dma_start(
                    logits_d[tbs, :], outL[:, :])

            ho_tiles = {}

            def fetch_other(li):
                Ho0 = work.tile([P, 4, D], f16, tag="Ho0", bufs=1, name=f"Ho0_{li}")
                Ho1 = work.tile([P, 4, D], f16, tag="Ho1", bufs=1, name=f"Ho1_{li}")
                for j in range(4):
                    nc.sync.dma_start(Ho0[:, j, :], ag_out[j * P:(j + 1) * P, :])
                    nc.sync.dma_start(Ho1[:, j, :], ag_out[HALF + j * P:HALF + (j + 1) * P, :])
                ho_tiles[li] = (Ho0, Ho1)

            def combine_other(li):
                Ho0, Ho1 = ho_tiles.pop(li - 1)
                tmp = work.tile([P, 4, D], f16, tag="HoT", bufs=1, name=f"HoT_{li}")
                nc.scalar.mul(tmp[:, :, :], Ho0[:, :, :], gsel_t[:, 1:2])
                nc.vector.scalar_tensor_tensor(
                    x16[:, 4:8, :], Ho1[:, :, :], gsel_t[:, 0:1], tmp[:, :, :],
                    OP.mult, OP.add)
                for j in range(4):
                    pe_transpose(x16[:, 4 + j, :], (4 + j) * P, f"o{li}{j}")

            def wave(wv, li):
                """x_sparse (fp16) + rope (fp16 DVE, fp8 out) + fp8 DR score
                strips for one column half."""
                LAGP = 4
                s_lo = wv * 4
                tsl = slice(wv * HALF, (wv + 1) * HALF)

                def sc_mms(ps_list, pcp):
                    psl = slice(2 * pcp, 2 * pcp + 2)
                    for si, psS in enumerate(ps_list):
                        s = s_lo + si
                        ssl = slice(s * P, (s + 1) * P)
                        osl = slice(si * P, HALF)  # causal trim (both waves)
                        nc.tensor.matmul(psS[:, osl], qrE[:, psl, ssl], qrE[:, psl, osl],
                                         start=(pcp == 0), stop=False,
                                         perf_mode=DR)
                        nc.tensor.matmul(psS[:, osl], qrO[:, psl, ssl], qrO[:, psl, osl],
                                         start=False, stop=(pcp == NPC // 2 - 1),
                                         perf_mode=DR)

                ps_list = []
                for si in range(4):
                    psS = psp.tile([P, HALF], f32, tag=f"sc{si}",
                                   name=f"psS{li}_{s_lo + si}")
                    ps_list.append(psS)
                for kk in range(NPC // 2):
                    if wv == 0:
                        Et2 = Eown[:, 2 * kk:2 * kk + 2, :]
                        Ot2 = Oown[:, 2 * kk:2 * kk + 2, :]
                    else:
                        Ew = work.tile([P, 2, HALF], f16, tag="E1")
                        Ow = work.tile([P, 2, HALF], f16, tag="O1")
                        Et2 = Ew[:, :, :]
                        Ot2 = Ow[:, :, :]
                    for k in range(2):
                        pc = 2 * kk + k
                        pcs = slice(pc * P, (pc + 1) * P)
                        psE = psp.tile([P, HALF], f32, tag="xspE", bufs=2)
                        psO = psp.tile([P, HALF], f32, tag="xspO", bufs=2)
                        if wv == 0 and kk < 3:
                            # split the first chunks by column half so PE can
                            # start as soon as the h0 epilogue lands, instead
                            # of gating on the h1 AllReduce readback
                            for ch in range(2):
                                csl2 = slice(ch * QUART, (ch + 1) * QUART)
                                nc.tensor.matmul(psE[:, csl2], wxe[:, 0, pcs],
                                                 xT[:, 0, csl2], start=True, stop=False)
                                nc.tensor.matmul(psE[:, csl2], wxe[0:64, 1, pcs],
                                                 xT[0:64, 1, csl2], start=False, stop=True)
                                nc.tensor.matmul(psO[:, csl2], wxo[:, 0, pcs],
                                                 xT[:, 0, csl2], start=True, stop=False)
                                nc.tensor.matmul(psO[:, csl2], wxo[0:64, 1, pcs],
                                                 xT[0:64, 1, csl2], start=False, stop=True)
                        else:
                            mm_pair(psE[:, :], wxe, pcs, xT, tsl)
                            mm_pair(psO[:, :], wxo, pcs, xT, tsl)
                        nc.scalar.activation(Et2[:, k, :], psE[:, :], AF.Relu)
                        nc.scalar.activation(Ot2[:, k, :], psO[:, :], AF.Relu)
                    psl2 = slice(2 * kk, 2 * kk + 2)
                    cs = cos_t[:, psl2, tsl]
                    sn = sin_t[:, psl2, tsl]
                    t1 = work.tile([P, 2, HALF], f16, tag="rt1")
                    t2 = work.tile([P, 2, HALF], f16, tag="rt2")
                    nc.vector.tensor_mul(t1[:, :, :], Et2, cs)
                    nc.vector.tensor_mul(t2[:, :, :], Ot2, sn)
                    nc.vector.tensor_sub(qrE[:, psl2, tsl], t1[:, :, :], t2[:, :, :])
                    t3 = work.tile([P, 2, HALF], f16, tag="rt3")
                    t4 = work.tile([P, 2, HALF], f16, tag="rt4")
                    nc.vector.tensor_mul(t3[:, :, :], Ot2, cs)
                    nc.vector.tensor_mul(t4[:, :, :], Et2, sn)
                    nc.vector.tensor_add(qrO[:, psl2, tsl], t3[:, :, :], t4[:, :, :])
                    if kk >= LAGP:
                        sc_mms(ps_list, kk - LAGP)
                    elif kk > 0:
                        # zero-stationary warmers: keep the PE clock ramped
                        # through the DVE-paced lead-in (adds 0 to a live
                        # strip accumulation, so results are unchanged)
                        for _w in range(WARM):
                            nc.tensor.matmul(ps_list[0][:, 0:HALF],
                                             zer8[:, :, 0:P],
                                             qrE[:, 0:2, 0:HALF],
                                             start=False, stop=False,
                                             perf_mode=DR)
                for pcp in range(NPC // 2 - LAGP, NPC // 2):
                    sc_mms(ps_list, pcp)
                for si in range(4):
                    s = s_lo + si
                    psS = ps_list[si]
                    dsl = slice(si * P, (si + 1) * P)
                    if wv == 0:
                        nc.vector.tensor_tensor(scT[:, s, dsl], psS[:, dsl],
                                                m0_t[:, :], OP.mult)
                    else:
                        sel = gsel_t[:, 0:1] if si % 2 == 1 else gsel_t[:, 1:2]
                        nc.scalar.mul(scT[:, s, dsl], psS[:, dsl], sel)
                    if si < 3:
                        csl = slice((si + 1) * P, HALF)
                        nc.scalar.copy(scT[:, s, csl], psS[:, csl])

            def ykv_block(tb, li):
                """scores @ x for own block tb, full LN, transpose to ykvT."""
                tbs = slice(tb * P, (tb + 1) * P)
                psY = psp.tile([P, D], f32, tag="xspE", bufs=2)
                s_list = [s for s in range(TBLK) if s % 4 <= tb]
                for s in s_list:
                    nc.tensor.matmul(psY[:, :], scT[:, s, tbs], x16[:, s, :],
                                     start=(s == s_list[0]), stop=(s == s_list[-1]))
                st = stp.tile([P, 6], f32, tag="bnst")
                nc.vector.bn_stats(st[:, :], psY[:, :])
                mv = stp.tile([P, 2], f32, tag="bnmv")
                nc.vector.bn_aggr(mv[:, :], st[:, :])
                sd = stp.tile([P, 1], f32, tag=f"sdk{tb}")
                nc.scalar.activation(sd[:, :], mv[:, 1:2], AF.Sqrt, bias=eps_t[:, :])
                rk = stp.tile([P, 1], f32, tag=f"rk{tb}")
                nc.vector.reciprocal(rk[:, :], sd[:, :])
                nmr = stp.tile([P, 1], f32, tag=f"nmk{tb}")
                nc.vector.scalar_tensor_tensor(nmr[:, :], mv[:, 0:1], -1.0, rk[:, :],
                                               OP.mult, OP.mult)
                ykvn = work.tile([P, D], f16, tag="ykvn", bufs=2)
                nc.scalar.activation(ykvn[:, :], psY[:, :], AF.Identity,
                                     bias=nmr[:, :], scale=rk[:, :])
                tp0 = psp.tile([P, P], f16, tag="xspO", bufs=2, name=f"ytp0_{li}{tb}")
                nc.tensor.transpose(tp0[:, :], ykvn[:, 0:P], ident_t[:, :])
                nc.scalar.copy(ykvT[:, 0, tbs], tp0[:, :])
                tp1 = psp.tile([P, P], f16, tag="xspO", bufs=2, name=f"ytp1_{li}{tb}")
                nc.tensor.transpose(tp1[0:64, :], ykvn[:, P:D], ident_t[:, :])
                nc.scalar.copy(ykvT[0:64, 1, tbs], tp1[0:64, :])

            def layer(li):
                wave(0, li)
                if li > 0:
                    combine_other(li)
                wave(1, li)

                for tb in range(4):
                    ykv_block(tb, li)

                # ---- y_sparse / xy / mlp in two own-column halves ----
                psM = []
                for _mi in range(4):
                    psM_t = psp.tile([P, D], f32, tag=f"sc{_mi}", name=f"psM{li}_{_mi}")
                    psM.append(psM_t)
                for h in range(2):
                    osl = slice(h * QUART, (h + 1) * QUART)
                    for side in range(2):
                        wa = wye if side == 0 else wyo
                        own = Eown if side == 0 else Oown
                        enc_t = ence_t if side == 0 else enco_t
                        for kk in range(NPC // 2):
                            ys2 = work.tile([P, 2, QUART], f16, tag="ys")
                            for k in range(2):
                                pc = 2 * kk + k
                                pcs = slice(pc * P, (pc + 1) * P)
                                psYS = psp.tile([P, QUART], f32, tag="xspE", bufs=2)
                                mm_pair(psYS[:, :], wa, pcs, ykvT, osl)
                                # split relus ACT/DVE: shortens the h phase,
                                # which gates this half's AllReduce trigger
                                if pc % 3 == 2:
                                    nc.vector.tensor_scalar_max(ys2[:, k, :],
                                                                psYS[:, :], 0.0)
                                else:
                                    nc.scalar.activation(ys2[:, k, :], psYS[:, :],
                                                         AF.Relu)
                            xy2 = work.tile([P, 2, QUART], f16, tag="xy", bufs=3)
                            nc.vector.tensor_mul(xy2[:, :, :],
                                                 own[:, 2 * kk:2 * kk + 2, osl],
                                                 ys2[:, :, :])
                            for k in range(2):
                                pc = 2 * kk + k
                                last = (side == 1 and pc == NPC - 1)
                                for j in range(2):
                                    tb = 2 * h + j
                                    nc.tensor.matmul(psM[tb][:, :],
                                                     xy2[:, k, j * P:(j + 1) * P],
                                                     enc_t[:, pc, :],
                                                     start=(side == 0 and pc == 0),
                                                     stop=last)
                    # AllReduce of this half's MLP partial over the 4 heads
                    for j in range(2):
                        tb = 2 * h + j
                        bA = work.tile([P, D], f16, tag="bA", bufs=2, name=f"bA{j}")
                        if j == 0:
                            nc.scalar.copy(bA[:, :], psM[tb][:, :])
                        else:
                            nc.vector.tensor_copy(bA[:, :], psM[tb][:, :])
                        (nc.sync if j == 0 else nc.scalar).dma_start(
                            ar_in[h][j * P:(j + 1) * P, :], bA[:, :])
                    nc.gpsimd.collective_compute(
                        "AllReduce", OP.add,
                        replica_groups=GROUPS_AR if (li + h) % 2 == 0 else GROUPS_AR[::-1],
                        ins=[ar_in[h].opt()],
                        outs=[ar_out[h].opt()],
                    )

                # ---- per-half readback, ln(ymlp), residual, ln, new x ----
                for h in range(2):
                    Hall = work.tile([P, 2, D], f16, tag=f"Hall{h}", bufs=1,
                                     name=f"Hall{li}_{h}")
                    rb_q = [nc.sync, nc.scalar]
                    for j in range(2):
                        rb_q[j].dma_start(Hall[:, j, :], ar_out[h][j * P:(j + 1) * P, :])
                    rv, nmr = bn_stats2(Hall[:, :, :], f"H{li}{h}")
                    t1a = work.tile([P, 2, D], f32, tag="t1a", bufs=2)
                    for j in range(2):
                        nc.scalar.activation(t1a[:, j, :], Hall[:, j, :], AF.Identity,
                                             bias=nmr[:, j:j + 1], scale=rv[:, j:j + 1])
                    XM = work.tile([P, 2, D], f16, tag=f"XM{h}", bufs=1,
                                   name=f"XM{li}_{h}")
                    nc.vector.tensor_add(XM[:, :, :], t1a[:, :, :],
                                         x16[:, 2 * h:2 * h + 2, :])
                    rv2, nm2 = bn_stats2(XM[:, :, :], f"X{li}{h}")
                    for j in range(2):
                        slot = 2 * h + j
                        nc.scalar.activation(x16[:, slot, :], XM[:, j, :], AF.Identity,
                                             bias=nm2[:, j:j + 1], scale=rv2[:, j:j + 1])
                        if li < N_LAYER - 1:
                            (nc.gpsimd if slot % 2 == 0 else nc.sync).dma_start(
                                ag_in[slot * P:(slot + 1) * P, :], x16[:, slot, :])
                        pe_transpose(x16[:, slot, :], slot * P, f"n{li}{slot}")
                        if li == N_LAYER - 1:
                            lm_head_tb(slot)

                # ---- pair-exchange of the finished own half ----
                if li < N_LAYER - 1:
                    nc.gpsimd.collective_compute(
                        "AllGather", OP.bypass,
                        replica_groups=GROUPS_AG[li % 4:] + GROUPS_AG[:li % 4],
                        ins=[ag_in.opt()],
                        outs=[ag_out.opt()],
                    )
                    fetch_other(li)

            for rep in range(repeat):
                for li in range(N_LAYER):
                    layer(li)

    nc.compile()
    _CACHE[key] = nc
    return nc


def _pack_ktiles16(w):
    """[D, C] -> [128, 2, C] fp16 k-tile layout (rows 0:128 | 128:192+pad)."""
    c = w.shape[1]
    out = np.zeros((P, 2, c), dtype=np.float16)
    out[:, 0, :] = w[0:P].astype(np.float16)
    out[0:64, 1, :] = w[P:D].astype(np.float16)
    return out.reshape(P, 2 * c)


def make_inputs(idx, decoder_x, decoder_y, encoder, embed, pos_emb, lm_head):
    """Host-side prep: per-core input dicts (core c = head c//2, group c%2)."""
    idx = np.asarray(idx)
    decoder_x = np.asarray(decoder_x, dtype=np.float32)
    decoder_y = np.asarray(decoder_y, dtype=np.float32)
    encoder = np.asarray(encoder, dtype=np.float32).reshape(NH, N, D)
    embed = np.asarray(embed, dtype=np.float32)
    pos_emb = np.asarray(pos_emb, dtype=np.float32)
    lm_head = np.asarray(lm_head, dtype=np.float32)

    x0f = _ln_np(embed[idx[0]] + pos_emb[:T]).astype(np.float32)

    freqs = _get_freqs(N)
    fpair = freqs[0::2]
    tt = np.arange(T, dtype=np.float32)
    m0 = np.triu(np.ones((P, P), np.float32), k=1).astype(np.float16)

    lmh2 = np.zeros((P, 2, VOCAB), np.float16)
    lmh2[:, 0, :] = lm_head[0:P].astype(np.float16)
    lmh2[0:64, 1, :] = lm_head[P:D].astype(np.float16)
    lmh2 = lmh2.reshape(P, 2 * VOCAB)

    in_maps = []
    for c in range(NCORES):
        h, g = c // 2, c % 2
        own = OWN_A if g == 0 else OWN_B
        peer = OWN_B if g == 0 else OWN_A
        tsel = np.concatenate([np.arange(b * P, (b + 1) * P) for b in own + peer])
        tperm = tt[tsel]
        ph = ((fpair[:, None] * tperm[None, :]).astype(np.float32) % 1.0) \
            * np.float32(2.0 * math.pi)
        gsel = np.zeros((P, 2), np.float32)
        gsel[:, 0] = 1.0 if g == 0 else 0.0
        gsel[:, 1] = 1.0 - gsel[:, 0]
        x0c = x0f[tsel]
        in_maps.append({
            "x0": x0c.astype(np.float16),
            "x0t": _pack_ktiles16(x0c.T),
            "wxe": _pack_ktiles16(np.ascontiguousarray(decoder_x[h][:, 0::2])),
            "wxo": _pack_ktiles16(np.ascontiguousarray(decoder_x[h][:, 1::2])),
            "wye": _pack_ktiles16(np.ascontiguousarray(decoder_y[h][:, 0::2])),
            "wyo": _pack_ktiles16(np.ascontiguousarray(decoder_y[h][:, 1::2])),
            "ence": np.ascontiguousarray(encoder[h][0::2]).astype(np.float16),
            "enco": np.ascontiguousarray(encoder[h][1::2]).astype(np.float16),
            "cosT": np.cos(ph.astype(np.float64)).astype(np.float16),
            "sinT": np.sin(ph.astype(np.float64)).astype(np.float16),
            "m0": m0,
            "gsel": gsel,
            "lmh2": lmh2,
            "ident": np.eye(P, dtype=np.float16),
        })
    return in_maps


def kernel(idx, decoder_x, decoder_y, encoder, embed, pos_emb, lm_head):
    from concourse.bass_utils import run_bass_kernel_spmd

    nc = build_program()
    in_maps = make_inputs(idx, decoder_x, decoder_y, encoder, embed, pos_emb,
                          lm_head)
    res = run_bass_kernel_spmd(nc, in_maps, list(range(NCORES)))
    return assemble_logits(res.results)


def assemble_logits(results):
    logits = np.empty((T, VOCAB), np.float32)
    for c in (0, 1):
        own = OWN_A if c % 2 == 0 else OWN_B
        sl = results[c]["logits"]
        for pos, b in enumerate(own):
            logits[b * P:(b + 1) * P] = sl[pos * P:(pos + 1) * P]
    return logits.reshape(1, T, VOCAB).astype(np.float32)
